# revision 37
# baseline (speedup 1.0000x reference)
"""
Trainium2 Bass kernel for nn_CIN (xDeepFM-style Compressed Interaction Network).

Reference computation (B=1024, F=39, D=32):
    x0 = x;  h = x
    for layer i in 0..2:
        z[b,d,:] = outer(x0[b,:,d], h[b,:,d]).flatten()     # (B, D, F*Hp)
        out = relu(z @ W_i + b_i)                           # (B, D, 256) -> (B, 256, D)
        h = out[:, :128]; finals.append(out[:, 128:])       # (last layer: all 256)
    res = concat(finals, 1).sum(-1) @ fc_w + fc_b           # (B, 1)

Strategy (data-parallel over 8 cores, 128 samples each):
  Everything on-chip lives TRANSPOSED: activations as [channel, (b,d)] so that
  - the matmul contraction (f*h) is on the partition axis (weights stationary),
  - the per-sample Khatri-Rao z-formation  z^T[f*128+h, bd] = x0^T[f,bd]*h^T[h,bd]
    is a plain VectorEngine tensor_tensor multiply against a host-precomputed
    128-partition broadcast of x0^T (per f),
  - bias+relu is a per-partition ScalarEngine activation (o on partitions).
  Layer-0's z is a pure function of the input x, so it is precomputed on host
  (symmetric Khatri-Rao of x0 with itself folded over (f,h)<->(h,f): K 1521->780,
  padded to 7 K-chunks) and streamed per bd-chunk.

  All three layers are fused per 512-wide bd-chunk: layer-0 of chunk c+1 and the
  first layer-1 f-block of chunk c+1 are issued between the layer transitions of
  chunk c so the PE always has ready matmuls while relu->z-formation chains
  resolve.  The final fc contraction is NOT done on the PE: the 4 "finals"
  halves are d-sum-reduced on the VectorEngine ([128,(16,32)]->[128,16]) and the
  tiny [512,128]@[512] fc matvec is applied on the host after gathering.
  All matmul inputs are bf16 (fp32 PSUM accumulation).
"""

import os
import sys

import numpy as np

for _p in ("/opt/trn_rl_repo",):
    if os.path.isdir(_p) and _p not in sys.path:
        sys.path.append(_p)

import ml_dtypes

import concourse.bass as bass
import concourse.mybir as mybir
import concourse.tile as tile
from concourse import bacc
from concourse.bass_utils import run_bass_kernel_spmd

BF16 = ml_dtypes.bfloat16

# Problem constants (hardcoded per contract).
B, F, D = 1024, 39, 32
O = 256            # per-layer conv output channels
NCORES = 8
BC = B // NCORES   # samples per core = 128
NBD = BC * D       # bd columns per core = 4096
CH = 512           # free-dim chunk width
NCH = NBD // CH    # 8 chunks
NB_CH = CH // D    # b's per chunk = 16
NP0 = F * (F + 1) // 2   # layer-0 folded symmetric pairs = 780
K0C = 7                  # layer-0 K chunks (780 padded to 896)
K0 = K0C * 128
FBLK = 8           # f's per x0-broadcast DMA block
FBLKS = [8, 8, 8, 8, 7]
NJ = 4             # finals halves: L0-hi, L1-hi, L2-lo, L2-hi

LAST_RESULT = None  # BassKernelResults of the most recent run (for test.py)
_CACHE = {}


def _build_program():
    """Build + compile the per-core Bass/Tile program (identical on all cores)."""
    nc = bacc.Bacc("TRN2", target_bir_lowering=False, debug=False)
    dt = mybir.dt

    # layer-0 z, per bd-chunk: z0t[c][p, k*CH+col] = z0[k*128+p, c*CH+col]
    z0t = nc.dram_tensor("z0t", [NCH, 128, K0C * CH], dt.bfloat16,
                         kind="ExternalInput").ap()
    # broadcast-x0, pre-arranged per (bd-chunk, f-block) so every DMA reads
    # partition-contiguous runs (descriptor-light on the HWDGE ring).
    # Chunk 0 is stored fp8-e4m3 and CAST TO BF16 IN-FLIGHT by the SDMA
    # engines (SWDGE cast-DMA): the DMA-bound head phase moves half the
    # bytes, while on-chip tiles stay bf16 so the DVE z-formation keeps full
    # speed (an fp8 DVE *source* runs ~1.9x slower -- measured and reverted).
    # Quantizing the x-factor of 1/8 of the batch costs rel_l2 ~1.4e-2 < 2e-2.
    x0b8 = nc.dram_tensor("x0b8", [len(FBLKS), 128, FBLK * CH], dt.float8e4,
                          kind="ExternalInput").ap()
    x0b = nc.dram_tensor("x0b", [NCH - 1, len(FBLKS), 128, FBLK * CH],
                         dt.bfloat16, kind="ExternalInput").ap()
    w0 = nc.dram_tensor("w0", [128, K0C * O], dt.bfloat16, kind="ExternalInput").ap()
    w1 = nc.dram_tensor("w1", [128, F * O], dt.bfloat16, kind="ExternalInput").ap()
    w2 = nc.dram_tensor("w2", [128, F * O], dt.bfloat16, kind="ExternalInput").ap()
    bia = nc.dram_tensor("bia", [128, 6], dt.float32, kind="ExternalInput").ap()
    # d-summed finals halves: res[p, c, j, i] = sum_d finals_j[p, (c*16+i)*32+d]
    # bf16: the 2-byte output keeps the DVE d-sum reduce in its 2x perf mode
    # (fp32 dest would force 1x) and the rounding (~0.2% of a 32-term sum) is
    # far below the bf16 matmul noise floor.
    res = nc.dram_tensor("res", [128, NCH, NJ, NB_CH], dt.bfloat16,
                         kind="ExternalOutput").ap()

    with tile.TileContext(nc) as tc:
        with (
            tc.tile_pool(name="consts", bufs=1) as consts,
            tc.tile_pool(name="z0p", bufs=2) as z0p,
            tc.tile_pool(name="xbp", bufs=8) as xbp,
            tc.tile_pool(name="zp", bufs=10) as zp,
            tc.tile_pool(name="hp", bufs=4) as hp,
            tc.tile_pool(name="fp", bufs=6) as fp,
            tc.tile_pool(name="rp", bufs=2) as rp,
            tc.tile_pool(name="psp", bufs=8, space="PSUM") as psp,
        ):
            w0_sb = consts.tile([128, K0C * O], dt.bfloat16)
            w1_sb = consts.tile([128, F * O], dt.bfloat16)
            w2_sb = consts.tile([128, F * O], dt.bfloat16)
            bia_sb = consts.tile([128, 6], dt.float32)

            # --- DMA helpers (all prefetch on the Sync HWDGE ring; the Scalar
            # ring is kept free of prefetches so relu epilogues never queue
            # behind descriptor-gen; per-chunk res output goes on the Scalar
            # ring AFTER that chunk's epilogues).
            def load_z0(c, eng=None):
                t = z0p.tile([128, K0C * CH], dt.bfloat16, name=f"z0_{c}", tag="z0")
                (eng or nc.sync).dma_start(t[:], z0t[c])
                return t

            def load_xb(c, bi):
                w = FBLKS[bi]
                t = xbp.tile([128, w * CH], dt.bfloat16, tag="xbt",
                             name=f"xbt_{c}_{bi}")
                if c == 0:
                    nc.gpsimd.dma_start(t[:], x0b8[bi, :, : w * CH])
                else:
                    nc.sync.dma_start(t[:], x0b[c - 1, bi, :, : w * CH])
                return t

            def wblock(w_sb, w_dram, blk):
                f0 = sum(FBLKS[:blk])
                w = FBLKS[blk]
                nc.sync.dma_start(w_sb[:, f0 * O:(f0 + w) * O],
                                  w_dram[:, f0 * O:(f0 + w) * O])

            def epilogue(ps, bias_col, dst):
                # dst = bf16(relu(psum + bias)), bias per-partition
                nc.scalar.activation(
                    dst, ps[:], mybir.ActivationFunctionType.Relu,
                    bias=bia_sb[:, bias_col:bias_col + 1], scale=1.0,
                )

            def dsum(f_t, res_t, j):
                # res_t[:, j, i] = sum_d f_t[:, i*32+d]
                with nc.allow_low_precision(reason="32-term bf16 d-sum"):
                    nc.vector.tensor_reduce(
                        res_t[:, j], f_t[:].rearrange("p (i d) -> p i d", d=D),
                        axis=mybir.AxisListType.X, op=mybir.AluOpType.add,
                    )

            def new_ps(nm):
                return [psp.tile([128, CH], dt.float32, name=f"{nm}_{hf}", tag="ps")
                        for hf in range(2)]

            def l0_mms(ps0, z0_t):
                for k in range(K0C):
                    for hf in range(2):
                        nc.tensor.matmul(
                            ps0[hf][:],
                            lhsT=w0_sb[:, k * O + hf * 128: k * O + hf * 128 + 128],
                            rhs=z0_t[:, k * CH: (k + 1) * CH],
                            start=(k == 0), stop=(k == K0C - 1),
                        )

            def blocks(blk_list):
                # (blk, j0, nf, f0) subgroups: 4-f DVE batches within f-blocks
                out = []
                for blk in blk_list:
                    bw = FBLKS[blk]
                    for j0, nf in ((0, 4), (4, bw - 4)):
                        out.append((blk, j0, nf, blk * FBLK + j0))
                return out

            def sub_zt(xbt, j0, nf, h_in):
                # One DVE op forms z^T for nf f's; the h operand is re-read via
                # a stride-0 AP dim.
                zt = zp.tile([128, 4 * CH], dt.bfloat16, tag="zt")
                nc.vector.tensor_mul(
                    zt[:].rearrange("p (f c) -> p f c", f=4)[:, :nf],
                    xbt[:, j0 * CH: (j0 + nf) * CH]
                        .rearrange("p (f c) -> p f c", f=nf),
                    h_in[:].unsqueeze(1).broadcast_to((128, nf, CH)),
                )
                return zt

            def sub_mms(w_sb, ps, zt, f0, nf, hfs=(0, 1)):
                for i in range(nf):
                    f = f0 + i
                    for hf in hfs:
                        nc.tensor.matmul(
                            ps[hf][:],
                            lhsT=w_sb[:, f * O + hf * 128: f * O + hf * 128 + 128],
                            rhs=zt[:, i * CH: (i + 1) * CH],
                            start=(f == 0), stop=(f == F - 1),
                        )

            def ll_part(li, ps, xbts, h_in, blk_list, subs=None):
                # TT z-formation + matmuls of layer li+1 for f-blocks blk_list
                w_sb = (w1_sb, w2_sb)[li]
                for blk, j0, nf, f0 in (subs if subs is not None
                                        else blocks(blk_list)):
                    zt = sub_zt(xbts[blk], j0, nf, h_in)
                    sub_mms(w_sb, ps, zt, f0, nf)

            def new_h(nm):
                return hp.tile([128, CH], dt.bfloat16, name=nm, tag="h")

            def new_f(nm):
                return fp.tile([128, CH], dt.bfloat16, name=nm, tag="f")

            # ---------------- startup: chunk 0 head ----------------
            # The head phase is DMA-rate-bound and the SDMA engines service
            # the HWDGE rings with no priority (a second ring's small pieces
            # can finish LAST behind bulk -- measured), so everything streams
            # on the single sync ring in strict first-use order at fine
            # granularity.
            nc.sync.dma_start(w0_sb[:, 0:O], w0[:, 0:O])        # L0 k=0 wts
            z0_t = z0p.tile([128, K0C * CH], dt.bfloat16, name="z0_0", tag="z0")
            nc.sync.dma_start(z0_t[:, 0:CH], z0t[0][:, 0:CH])   # L0 k=0 cols
            nc.sync.dma_start(z0_t[:, CH:], z0t[0][:, CH:])
            nc.sync.dma_start(w0_sb[:, O:], w0[:, O:])
            nc.sync.dma_start(bia_sb[:], bia)
            xbts_c = []
            for bi in range(len(FBLKS)):                        # L1(0) blocks,
                xbts_c.append(load_xb(0, bi))                   # paired with
                wblock(w1_sb, w1, bi)                           # their weights

            ps0 = new_ps("ps0_0")
            l0_mms(ps0, z0_t)

            h1 = new_h("h1_0")
            f0_t = new_f("f0_0")
            epilogue(ps0[0], 0, h1[:])
            epilogue(ps0[1], 1, f0_t[:])
            res_t = rp.tile([128, NJ, NB_CH], dt.bfloat16, name="res_0", tag="res")
            dsum(f0_t, res_t, 0)

            ps1 = new_ps("ps1_0")
            ll_part(0, ps1, xbts_c, h1, [0])
            z0_n = load_z0(1)
            for blk in range(len(FBLKS)):
                wblock(w2_sb, w2, blk)

            # ---------------- main loop over bd-chunks ----------------
            # c == NCH-2: the last f-subgroup of layer 2 is deferred into the
            #   next iteration, where it fills the L1->L2 transition gap that
            #   chunk NCH-1 cannot cover with a next-chunk layer-0.
            # c == NCH-1: layer 2's last f-block runs hf0-then-hf1 so the f2a
            #   epilogue+dsum chain hides under the hf1 matmuls.
            defer = None
            for c in range(NCH):
                last, pen = c == NCH - 1, c == NCH - 2
                # rest of layer 1 for chunk c
                ll_part(0, ps1, xbts_c, h1, list(range(1, len(FBLKS))))
                h2 = new_h(f"h2_{c}")
                f1_t = new_f(f"f1_{c}")
                epilogue(ps1[0], 2, h2[:])
                if not last:
                    # layer 0 of chunk c+1 fills the relu->z2 latency
                    ps0_n = new_ps(f"ps0_{c + 1}")
                    l0_mms(ps0_n, z0_n)
                else:
                    # deferred last subgroup of chunk c-1's layer 2
                    d_ps2, d_xbt, d_h2, d_res = defer
                    zt_d = sub_zt(d_xbt, 4, 3, d_h2)
                    sub_mms(w2_sb, d_ps2, zt_d, F - 3, 3)
                    f2a_d = new_f(f"f2a_{c - 1}")
                    f2b_d = new_f(f"f2b_{c - 1}")
                    epilogue(d_ps2[0], 4, f2a_d[:])
                    epilogue(d_ps2[1], 5, f2b_d[:])
                epilogue(ps1[1], 3, f1_t[:])
                # layer 2 for chunk c
                ps2 = new_ps(f"ps2_{c}")
                if last:
                    ll_part(1, ps2, xbts_c, h2, [0])
                    dsum(f2a_d, d_res, 2)
                    dsum(f2b_d, d_res, 3)
                    nc.sync.dma_start(res[:, c - 1], d_res[:])
                    ll_part(1, ps2, xbts_c, h2, [1, 2, 3])
                    # last f-block: all hf0 (stops ps2[0]), then all hf1
                    zt_a = sub_zt(xbts_c[4], 0, 4, h2)
                    zt_b = sub_zt(xbts_c[4], 4, 3, h2)
                    sub_mms(w2_sb, ps2, zt_a, 4 * FBLK, 4, hfs=(0,))
                    sub_mms(w2_sb, ps2, zt_b, 4 * FBLK + 4, 3, hfs=(0,))
                    f2a = new_f(f"f2a_{c}")
                    epilogue(ps2[0], 4, f2a[:])
                    dsum(f1_t, res_t, 1)
                    dsum(f2a, res_t, 2)
                    # j=0..2 slots go out under the hf1 matmuls; only the 8KB
                    # j=3 piece remains on the final dependency chain
                    nc.sync.dma_start(res[:, c, 0:3], res_t[:, 0:3])
                    sub_mms(w2_sb, ps2, zt_a, 4 * FBLK, 4, hfs=(1,))
                    sub_mms(w2_sb, ps2, zt_b, 4 * FBLK + 4, 3, hfs=(1,))
                    # final chain, pipelined in column halves: relu of half 1
                    # overlaps the d-sum of half 0
                    f2b = new_f(f"f2b_{c}")
                    HB = CH // 2
                    for hc in range(2):
                        sl = slice(hc * HB, (hc + 1) * HB)
                        nc.scalar.activation(
                            f2b[:, sl], ps2[1][:, sl],
                            mybir.ActivationFunctionType.Relu,
                            bias=bia_sb[:, 5:6], scale=1.0,
                        )
                        with nc.allow_low_precision(reason="32-term bf16 d-sum"):
                            nc.vector.tensor_reduce(
                                res_t[:, 3, hc * 8:(hc + 1) * 8],
                                f2b[:, sl].rearrange("p (i d) -> p i d", d=D),
                                axis=mybir.AxisListType.X, op=mybir.AluOpType.add,
                            )
                    nc.sync.dma_start(res[:, c, 3:4], res_t[:, 3:4])
                    break
                elif pen:
                    ll_part(1, ps2, xbts_c, h2,
                            blk_list=None, subs=blocks([0, 1, 2, 3])
                            + [(4, 0, 4, 4 * FBLK)])
                    defer = (ps2, xbts_c[4], h2, res_t)
                else:
                    ll_part(1, ps2, xbts_c, h2, list(range(len(FBLKS))))
                h1_n = new_h(f"h1_{c + 1}")
                f0_n = new_f(f"f0_{c + 1}")
                epilogue(ps0_n[0], 0, h1_n[:])
                epilogue(ps0_n[1], 1, f0_n[:])
                res_n = rp.tile([128, NJ, NB_CH], dt.bfloat16,
                                name=f"res_{c + 1}", tag="res")
                # next chunk's x0b + first layer-1 f-block
                xbt0_n = load_xb(c + 1, 0)
                ps1_n = new_ps(f"ps1_{c + 1}")
                ll_part(0, ps1_n, [xbt0_n], h1_n, [0])
                if c < NCH - 2:
                    z0_nn = load_z0(c + 2)
                # finals epilogues + d-sums for chunk c
                dsum(f1_t, res_t, 1)
                dsum(f0_n, res_n, 0)
                if not pen:
                    f2a = new_f(f"f2a_{c}")
                    f2b = new_f(f"f2b_{c}")
                    epilogue(ps2[0], 4, f2a[:])
                    epilogue(ps2[1], 5, f2b[:])
                    dsum(f2a, res_t, 2)
                    dsum(f2b, res_t, 3)
                    # res out on the Scalar ring, after this chunk's epilogues
                    nc.scalar.dma_start(res[:, c], res_t[:])
                xbts_c = [xbt0_n] + [load_xb(c + 1, bi)
                                     for bi in range(1, len(FBLKS))]
                h1, ps1, res_t = h1_n, ps1_n, res_n
                z0_n = z0_nn if c < NCH - 2 else None

    nc.compile()
    return nc


def _prep_inputs(x, W0, b0, W1, b1, W2, b2, fc_w, fc_b):
    """Host-side preprocessing -> per-core input maps (numpy only)."""
    x = np.asarray(x, dtype=np.float32)
    xT = np.ascontiguousarray(x.transpose(1, 0, 2)).reshape(F, B * D)  # [39, B*D]
    xTb = xT.astype(BF16)

    # Layer-0: z0 = KhatriRao(x0,x0) is symmetric in (f,h), so fold the weights
    # over (f,h)<->(h,f) and keep only the f<=h pairs: K 1521 -> 780 (pad 896).
    fi, hi = np.triu_indices(F)                       # 780 pairs, f<=h
    z0 = (xTb[fi, :].astype(np.float32)
          * xTb[hi, :].astype(np.float32)).astype(BF16)   # [780, B*D]
    z0p = np.zeros((K0, B * D), dtype=BF16)
    z0p[:NP0] = z0

    W0m = np.asarray(W0, dtype=np.float32).reshape(F, F, O)
    W0f = W0m[fi, hi] + np.where(fi != hi, 1.0, 0.0)[:, None] * W0m[hi, fi]

    def wdev(W, kb):
        Wb = np.zeros((kb * 128, O), dtype=np.float32)
        Wb[: W.shape[0]] = np.asarray(W, dtype=np.float32)
        # [kb*128, O] -> [128(h), kb*O] with layout w[h, k, o] = W[k*128+h, o]
        return np.ascontiguousarray(
            Wb.reshape(kb, 128, O).transpose(1, 0, 2)).reshape(128, kb * O).astype(BF16)

    w0d, w1d, w2d = wdev(W0f, K0C), wdev(W1, F), wdev(W2, F)
    bia = np.stack([np.asarray(b, dtype=np.float32)[hf * 128:(hf + 1) * 128]
                    for b in (b0, b1, b2) for hf in range(2)], axis=1)
    bia = np.ascontiguousarray(bia, dtype=np.float32)            # [128, 6]

    in_maps = []
    for c in range(NCORES):
        sl = np.s_[:, c * NBD:(c + 1) * NBD]
        xc = np.asarray(xTb[sl])                                  # [39, 4096]
        # -> [chunk, block, partition, f_local*CH]; every partition holds the
        # same row (broadcast), block 4 zero-padded from 7 to 8 f's
        xr = np.zeros((NCH, len(FBLKS), FBLK, CH), dtype=BF16)
        xr_v = xc.reshape(F, NCH, CH).transpose(1, 0, 2)          # [8, 39, 512]
        xr.reshape(NCH, len(FBLKS) * FBLK, CH)[:, :F] = xr_v
        x0bb = np.broadcast_to(xr.reshape(NCH, len(FBLKS), 1, FBLK * CH),
                               (NCH, len(FBLKS), 128, FBLK * CH))
        x0b8c = np.ascontiguousarray(x0bb[0]).astype(ml_dtypes.float8_e4m3)
        x0bc = np.ascontiguousarray(x0bb[1:])
        # z0 for this core, per bd-chunk: [NCH, 128, K0C*CH]
        z0c = np.ascontiguousarray(
            z0p[sl].reshape(K0C, 128, NCH, CH).transpose(2, 1, 0, 3)
            .reshape(NCH, 128, K0C * CH))
        in_maps.append({
            "z0t": z0c, "x0b8": x0b8c, "x0b": x0bc, "w0": w0d, "w1": w1d,
            "w2": w2d, "bia": bia,
        })
    return in_maps


def _postprocess_core(res_c, fc_w):
    """res_c [128, NCH, NJ, NB_CH] fp32 -> per-core output [BC] (no fc_b)."""
    fcw4 = np.asarray(fc_w, dtype=np.float32).reshape(NJ, 128)   # [j, p]
    r = np.asarray(res_c, dtype=np.float32)
    return np.einsum('pcji,jp->ci', r, fcw4).reshape(BC)


def kernel(x, W0, b0, W1, b1, W2, b2, fc_w, fc_b):
    global LAST_RESULT
    if "nc" not in _CACHE:
        _CACHE["nc"] = _build_program()
    nc = _CACHE["nc"]

    in_maps = _prep_inputs(x, W0, b0, W1, b1, W2, b2, fc_w, fc_b)
    trace = bool(int(os.environ.get("NN_CIN_TRACE", "0")))
    res = run_bass_kernel_spmd(nc, in_maps, core_ids=list(range(NCORES)), trace=trace)
    LAST_RESULT = res

    fcb = float(np.asarray(fc_b, dtype=np.float32).reshape(-1)[0])
    out = np.empty((B, 1), dtype=np.float32)
    for c in range(NCORES):
        out[c * BC:(c + 1) * BC, 0] = _postprocess_core(
            res.results[c]["res"], fc_w) + fcb
    return out


# revision 41
# speedup vs baseline: 1.0419x; 1.0419x over previous
"""
Trainium2 Bass kernel for nn_CIN (xDeepFM-style Compressed Interaction Network).

Reference computation (B=1024, F=39, D=32):
    x0 = x;  h = x
    for layer i in 0..2:
        z[b,d,:] = outer(x0[b,:,d], h[b,:,d]).flatten()     # (B, D, F*Hp)
        out = relu(z @ W_i + b_i)                           # (B, D, 256) -> (B, 256, D)
        h = out[:, :128]; finals.append(out[:, 128:])       # (last layer: all 256)
    res = concat(finals, 1).sum(-1) @ fc_w + fc_b           # (B, 1)

Strategy (data-parallel over 8 cores, 128 samples each):
  Everything on-chip lives TRANSPOSED: activations as [channel, (b,d)] so that
  - the matmul contraction (f*h) is on the partition axis (weights stationary),
  - the per-sample Khatri-Rao z-formation  z^T[f*128+h, bd] = x0^T[f,bd]*h^T[h,bd]
    is a plain VectorEngine tensor_tensor multiply against a host-precomputed
    128-partition broadcast of x0^T (per f),
  - bias+relu is a per-partition ScalarEngine activation (o on partitions).
  Layer-0's z is a pure function of the input x, so it is precomputed on host
  (symmetric Khatri-Rao of x0 with itself folded over (f,h)<->(h,f): K 1521->780,
  padded to 7 K-chunks) and streamed per bd-chunk.

  All three layers are fused per 512-wide bd-chunk: layer-0 of chunk c+1 and the
  first layer-1 f-block of chunk c+1 are issued between the layer transitions of
  chunk c so the PE always has ready matmuls while relu->z-formation chains
  resolve.  The final fc contraction is NOT done on the PE: the 4 "finals"
  halves are d-sum-reduced on the VectorEngine ([128,(16,32)]->[128,16]) and the
  tiny [512,128]@[512] fc matvec is applied on the host after gathering.
  All matmul inputs are bf16 (fp32 PSUM accumulation).
"""

import os
import sys

import numpy as np

for _p in ("/opt/trn_rl_repo",):
    if os.path.isdir(_p) and _p not in sys.path:
        sys.path.append(_p)

import ml_dtypes

import concourse.bass as bass
import concourse.mybir as mybir
import concourse.tile as tile
from concourse import bacc
from concourse.bass_utils import run_bass_kernel_spmd

BF16 = ml_dtypes.bfloat16

# Problem constants (hardcoded per contract).
B, F, D = 1024, 39, 32
O = 256            # per-layer conv output channels
NCORES = 8
BC = B // NCORES   # samples per core = 128
NBD = BC * D       # bd columns per core = 4096
CH = 512           # free-dim chunk width
NCH = NBD // CH    # 8 chunks
NB_CH = CH // D    # b's per chunk = 16
NP0 = F * (F + 1) // 2   # layer-0 folded symmetric pairs = 780
K0C = 7                  # layer-0 K chunks (780 padded to 896)
K0 = K0C * 128
FBLK = 8           # f's per x0-broadcast DMA block
FBLKS = [8, 8, 8, 8, 7]
NJ = 4             # finals halves: L0-hi, L1-hi, L2-lo, L2-hi

LAST_RESULT = None  # BassKernelResults of the most recent run (for test.py)
_CACHE = {}


def _build_program():
    """Build + compile the per-core Bass/Tile program (identical on all cores)."""
    nc = bacc.Bacc("TRN2", target_bir_lowering=False, debug=False)
    dt = mybir.dt

    # layer-0 z, per bd-chunk: z0t[c][p, k*CH+col] = z0[k*128+p, c*CH+col]
    z0t = nc.dram_tensor("z0t", [NCH, 128, K0C * CH], dt.bfloat16,
                         kind="ExternalInput").ap()
    # broadcast-x0, pre-arranged per (bd-chunk, f-block) so every DMA reads
    # partition-contiguous runs (descriptor-light on the HWDGE ring).
    # fp8 variants for the head chunk were tried and reverted: an fp8 DVE
    # source runs ~1.9x slower, and SWDGE cast-DMA (fp8 in HBM, bf16 in SBUF)
    # delivers far below HWDGE line rate -- both cost more than the bytes save.
    x0b = nc.dram_tensor("x0b", [NCH, len(FBLKS), 128, FBLK * CH],
                         dt.bfloat16, kind="ExternalInput").ap()
    w0 = nc.dram_tensor("w0", [128, K0C * O], dt.bfloat16, kind="ExternalInput").ap()
    w1 = nc.dram_tensor("w1", [128, F * O], dt.bfloat16, kind="ExternalInput").ap()
    w2 = nc.dram_tensor("w2", [128, F * O], dt.bfloat16, kind="ExternalInput").ap()
    bia = nc.dram_tensor("bia", [128, 6], dt.float32, kind="ExternalInput").ap()
    # d-summed finals halves: res[p, c, j, i] = sum_d finals_j[p, (c*16+i)*32+d]
    # bf16: the 2-byte output keeps the DVE d-sum reduce in its 2x perf mode
    # (fp32 dest would force 1x) and the rounding (~0.2% of a 32-term sum) is
    # far below the bf16 matmul noise floor.
    res = nc.dram_tensor("res", [128, NCH, NJ, NB_CH], dt.bfloat16,
                         kind="ExternalOutput").ap()

    with tile.TileContext(nc) as tc:
        with (
            tc.tile_pool(name="consts", bufs=1) as consts,
            tc.tile_pool(name="z0p", bufs=2) as z0p,
            tc.tile_pool(name="xbp", bufs=8) as xbp,
            tc.tile_pool(name="zp", bufs=10) as zp,
            tc.tile_pool(name="hp", bufs=4) as hp,
            tc.tile_pool(name="fp", bufs=6) as fp,
            tc.tile_pool(name="rp", bufs=2) as rp,
            tc.tile_pool(name="psp", bufs=8, space="PSUM") as psp,
        ):
            w0_sb = consts.tile([128, K0C * O], dt.bfloat16)
            w1_sb = consts.tile([128, F * O], dt.bfloat16)
            w2_sb = consts.tile([128, F * O], dt.bfloat16)
            bia_sb = consts.tile([128, 6], dt.float32)

            # --- DMA helpers (all prefetch on the Sync HWDGE ring; the Scalar
            # ring is kept free of prefetches so relu epilogues never queue
            # behind descriptor-gen; per-chunk res output goes on the Scalar
            # ring AFTER that chunk's epilogues).
            def load_z0(c, eng=None):
                t = z0p.tile([128, K0C * CH], dt.bfloat16, name=f"z0_{c}", tag="z0")
                (eng or nc.sync).dma_start(t[:], z0t[c])
                return t

            def load_xb(c, bi):
                w = FBLKS[bi]
                t = xbp.tile([128, w * CH], dt.bfloat16, tag="xbt",
                             name=f"xbt_{c}_{bi}")
                nc.sync.dma_start(t[:], x0b[c, bi, :, : w * CH])
                return t

            def wblock(w_sb, w_dram, blk):
                f0 = sum(FBLKS[:blk])
                w = FBLKS[blk]
                nc.sync.dma_start(w_sb[:, f0 * O:(f0 + w) * O],
                                  w_dram[:, f0 * O:(f0 + w) * O])

            def epilogue(ps, bias_col, dst):
                # dst = bf16(relu(psum + bias)), bias per-partition
                nc.scalar.activation(
                    dst, ps[:], mybir.ActivationFunctionType.Relu,
                    bias=bia_sb[:, bias_col:bias_col + 1], scale=1.0,
                )

            def dsum(f_t, res_t, j):
                # res_t[:, j, i] = sum_d f_t[:, i*32+d]
                with nc.allow_low_precision(reason="32-term bf16 d-sum"):
                    nc.vector.tensor_reduce(
                        res_t[:, j], f_t[:].rearrange("p (i d) -> p i d", d=D),
                        axis=mybir.AxisListType.X, op=mybir.AluOpType.add,
                    )

            def new_ps(nm):
                return [psp.tile([128, CH], dt.float32, name=f"{nm}_{hf}", tag="ps")
                        for hf in range(2)]

            def l0_mms(ps0, z0_t):
                for k in range(K0C):
                    for hf in range(2):
                        nc.tensor.matmul(
                            ps0[hf][:],
                            lhsT=w0_sb[:, k * O + hf * 128: k * O + hf * 128 + 128],
                            rhs=z0_t[:, k * CH: (k + 1) * CH],
                            start=(k == 0), stop=(k == K0C - 1),
                        )

            def blocks(blk_list):
                # (blk, j0, nf, f0) subgroups: 4-f DVE batches within f-blocks
                out = []
                for blk in blk_list:
                    bw = FBLKS[blk]
                    for j0, nf in ((0, 4), (4, bw - 4)):
                        out.append((blk, j0, nf, blk * FBLK + j0))
                return out

            def sub_zt(xbt, j0, nf, h_in):
                # One DVE op forms z^T for nf f's; the h operand is re-read via
                # a stride-0 AP dim.
                zt = zp.tile([128, 4 * CH], dt.bfloat16, tag="zt")
                nc.vector.tensor_mul(
                    zt[:].rearrange("p (f c) -> p f c", f=4)[:, :nf],
                    xbt[:, j0 * CH: (j0 + nf) * CH]
                        .rearrange("p (f c) -> p f c", f=nf),
                    h_in[:].unsqueeze(1).broadcast_to((128, nf, CH)),
                )
                return zt

            def sub_mms(w_sb, ps, zt, f0, nf, hfs=(0, 1)):
                for i in range(nf):
                    f = f0 + i
                    for hf in hfs:
                        nc.tensor.matmul(
                            ps[hf][:],
                            lhsT=w_sb[:, f * O + hf * 128: f * O + hf * 128 + 128],
                            rhs=zt[:, i * CH: (i + 1) * CH],
                            start=(f == 0), stop=(f == F - 1),
                        )

            def ll_part(li, ps, xbts, h_in, blk_list, subs=None):
                # TT z-formation + matmuls of layer li+1 for f-blocks blk_list
                w_sb = (w1_sb, w2_sb)[li]
                for blk, j0, nf, f0 in (subs if subs is not None
                                        else blocks(blk_list)):
                    zt = sub_zt(xbts[blk], j0, nf, h_in)
                    sub_mms(w_sb, ps, zt, f0, nf)

            def new_h(nm):
                return hp.tile([128, CH], dt.bfloat16, name=nm, tag="h")

            def new_f(nm):
                return fp.tile([128, CH], dt.bfloat16, name=nm, tag="f")

            # ---------------- startup: chunk 0 head ----------------
            # The head phase is DMA-rate-bound and the SDMA engines service
            # the HWDGE rings with no priority (a second ring's small pieces
            # can finish LAST behind bulk -- measured), so everything streams
            # on the single sync ring in strict first-use order at fine
            # granularity.
            nc.sync.dma_start(w0_sb[:, 0:O], w0[:, 0:O])        # L0 k=0 wts
            z0_t = z0p.tile([128, K0C * CH], dt.bfloat16, name="z0_0", tag="z0")
            nc.sync.dma_start(z0_t[:, 0:CH], z0t[0][:, 0:CH])   # L0 k=0 cols
            nc.sync.dma_start(z0_t[:, CH:], z0t[0][:, CH:])
            nc.sync.dma_start(w0_sb[:, O:], w0[:, O:])
            nc.sync.dma_start(bia_sb[:], bia)
            xbts_c = []
            for bi in range(len(FBLKS)):                        # L1(0) blocks,
                xbts_c.append(load_xb(0, bi))                   # paired with
                wblock(w1_sb, w1, bi)                           # their weights

            ps0 = new_ps("ps0_0")
            l0_mms(ps0, z0_t)

            h1 = new_h("h1_0")
            f0_t = new_f("f0_0")
            epilogue(ps0[0], 0, h1[:])
            epilogue(ps0[1], 1, f0_t[:])
            res_t = rp.tile([128, NJ, NB_CH], dt.bfloat16, name="res_0", tag="res")
            dsum(f0_t, res_t, 0)

            ps1 = new_ps("ps1_0")
            ll_part(0, ps1, xbts_c, h1, [0])
            z0_n = load_z0(1)
            for blk in range(len(FBLKS)):
                wblock(w2_sb, w2, blk)

            # ---------------- main loop over bd-chunks ----------------
            # c == NCH-2: the last f-subgroup of layer 2 is deferred into the
            #   next iteration, where it fills the L1->L2 transition gap that
            #   chunk NCH-1 cannot cover with a next-chunk layer-0.
            # c == NCH-1: layer 2's last f-block runs hf0-then-hf1 so the f2a
            #   epilogue+dsum chain hides under the hf1 matmuls.
            defer = None
            for c in range(NCH):
                last, pen = c == NCH - 1, c == NCH - 2
                # rest of layer 1 for chunk c
                ll_part(0, ps1, xbts_c, h1, list(range(1, len(FBLKS))))
                h2 = new_h(f"h2_{c}")
                f1_t = new_f(f"f1_{c}")
                epilogue(ps1[0], 2, h2[:])
                if not last:
                    # layer 0 of chunk c+1 fills the relu->z2 latency
                    ps0_n = new_ps(f"ps0_{c + 1}")
                    l0_mms(ps0_n, z0_n)
                else:
                    # deferred last subgroup of chunk c-1's layer 2
                    d_ps2, d_xbt, d_h2, d_res = defer
                    zt_d = sub_zt(d_xbt, 4, 3, d_h2)
                    sub_mms(w2_sb, d_ps2, zt_d, F - 3, 3)
                    f2a_d = new_f(f"f2a_{c - 1}")
                    f2b_d = new_f(f"f2b_{c - 1}")
                    epilogue(d_ps2[0], 4, f2a_d[:])
                    epilogue(d_ps2[1], 5, f2b_d[:])
                epilogue(ps1[1], 3, f1_t[:])
                # layer 2 for chunk c
                ps2 = new_ps(f"ps2_{c}")
                if last:
                    ll_part(1, ps2, xbts_c, h2, [0])
                    dsum(f2a_d, d_res, 2)
                    dsum(f2b_d, d_res, 3)
                    nc.sync.dma_start(res[:, c - 1], d_res[:])
                    ll_part(1, ps2, xbts_c, h2, [1, 2, 3])
                    # last f-block: all hf0 (stops ps2[0]), then all hf1
                    zt_a = sub_zt(xbts_c[4], 0, 4, h2)
                    zt_b = sub_zt(xbts_c[4], 4, 3, h2)
                    sub_mms(w2_sb, ps2, zt_a, 4 * FBLK, 4, hfs=(0,))
                    sub_mms(w2_sb, ps2, zt_b, 4 * FBLK + 4, 3, hfs=(0,))
                    f2a = new_f(f"f2a_{c}")
                    epilogue(ps2[0], 4, f2a[:])
                    dsum(f1_t, res_t, 1)
                    dsum(f2a, res_t, 2)
                    # j=0..2 slots go out under the hf1 matmuls; only the 8KB
                    # j=3 piece remains on the final dependency chain
                    nc.sync.dma_start(res[:, c, 0:3], res_t[:, 0:3])
                    sub_mms(w2_sb, ps2, zt_a, 4 * FBLK, 4, hfs=(1,))
                    sub_mms(w2_sb, ps2, zt_b, 4 * FBLK + 4, 3, hfs=(1,))
                    # final chain, pipelined in column halves: relu of half 1
                    # overlaps the d-sum of half 0
                    f2b = new_f(f"f2b_{c}")
                    HB = CH // 2
                    for hc in range(2):
                        sl = slice(hc * HB, (hc + 1) * HB)
                        nc.scalar.activation(
                            f2b[:, sl], ps2[1][:, sl],
                            mybir.ActivationFunctionType.Relu,
                            bias=bia_sb[:, 5:6], scale=1.0,
                        )
                        with nc.allow_low_precision(reason="32-term bf16 d-sum"):
                            nc.vector.tensor_reduce(
                                res_t[:, 3, hc * 8:(hc + 1) * 8],
                                f2b[:, sl].rearrange("p (i d) -> p i d", d=D),
                                axis=mybir.AxisListType.X, op=mybir.AluOpType.add,
                            )
                    nc.sync.dma_start(res[:, c, 3:4], res_t[:, 3:4])
                    break
                elif pen:
                    ll_part(1, ps2, xbts_c, h2,
                            blk_list=None, subs=blocks([0, 1, 2, 3])
                            + [(4, 0, 4, 4 * FBLK)])
                    defer = (ps2, xbts_c[4], h2, res_t)
                else:
                    ll_part(1, ps2, xbts_c, h2, list(range(len(FBLKS))))
                h1_n = new_h(f"h1_{c + 1}")
                f0_n = new_f(f"f0_{c + 1}")
                epilogue(ps0_n[0], 0, h1_n[:])
                epilogue(ps0_n[1], 1, f0_n[:])
                res_n = rp.tile([128, NJ, NB_CH], dt.bfloat16,
                                name=f"res_{c + 1}", tag="res")
                # next chunk's x0b + first layer-1 f-block
                xbt0_n = load_xb(c + 1, 0)
                ps1_n = new_ps(f"ps1_{c + 1}")
                ll_part(0, ps1_n, [xbt0_n], h1_n, [0])
                if c < NCH - 2:
                    z0_nn = load_z0(c + 2)
                # finals epilogues + d-sums for chunk c
                dsum(f1_t, res_t, 1)
                dsum(f0_n, res_n, 0)
                if not pen:
                    f2a = new_f(f"f2a_{c}")
                    f2b = new_f(f"f2b_{c}")
                    epilogue(ps2[0], 4, f2a[:])
                    epilogue(ps2[1], 5, f2b[:])
                    dsum(f2a, res_t, 2)
                    dsum(f2b, res_t, 3)
                    # res out on the Scalar ring, after this chunk's epilogues
                    nc.scalar.dma_start(res[:, c], res_t[:])
                xbts_c = [xbt0_n] + [load_xb(c + 1, bi)
                                     for bi in range(1, len(FBLKS))]
                h1, ps1, res_t = h1_n, ps1_n, res_n
                z0_n = z0_nn if c < NCH - 2 else None

    nc.compile()
    return nc


def _prep_inputs(x, W0, b0, W1, b1, W2, b2, fc_w, fc_b):
    """Host-side preprocessing -> per-core input maps (numpy only)."""
    x = np.asarray(x, dtype=np.float32)
    xT = np.ascontiguousarray(x.transpose(1, 0, 2)).reshape(F, B * D)  # [39, B*D]
    xTb = xT.astype(BF16)

    # Layer-0: z0 = KhatriRao(x0,x0) is symmetric in (f,h), so fold the weights
    # over (f,h)<->(h,f) and keep only the f<=h pairs: K 1521 -> 780 (pad 896).
    fi, hi = np.triu_indices(F)                       # 780 pairs, f<=h
    z0 = (xTb[fi, :].astype(np.float32)
          * xTb[hi, :].astype(np.float32)).astype(BF16)   # [780, B*D]
    z0p = np.zeros((K0, B * D), dtype=BF16)
    z0p[:NP0] = z0

    W0m = np.asarray(W0, dtype=np.float32).reshape(F, F, O)
    W0f = W0m[fi, hi] + np.where(fi != hi, 1.0, 0.0)[:, None] * W0m[hi, fi]

    def wdev(W, kb):
        Wb = np.zeros((kb * 128, O), dtype=np.float32)
        Wb[: W.shape[0]] = np.asarray(W, dtype=np.float32)
        # [kb*128, O] -> [128(h), kb*O] with layout w[h, k, o] = W[k*128+h, o]
        return np.ascontiguousarray(
            Wb.reshape(kb, 128, O).transpose(1, 0, 2)).reshape(128, kb * O).astype(BF16)

    w0d, w1d, w2d = wdev(W0f, K0C), wdev(W1, F), wdev(W2, F)
    bia = np.stack([np.asarray(b, dtype=np.float32)[hf * 128:(hf + 1) * 128]
                    for b in (b0, b1, b2) for hf in range(2)], axis=1)
    bia = np.ascontiguousarray(bia, dtype=np.float32)            # [128, 6]

    in_maps = []
    for c in range(NCORES):
        sl = np.s_[:, c * NBD:(c + 1) * NBD]
        xc = np.asarray(xTb[sl])                                  # [39, 4096]
        # -> [chunk, block, partition, f_local*CH]; every partition holds the
        # same row (broadcast), block 4 zero-padded from 7 to 8 f's
        xr = np.zeros((NCH, len(FBLKS), FBLK, CH), dtype=BF16)
        xr_v = xc.reshape(F, NCH, CH).transpose(1, 0, 2)          # [8, 39, 512]
        xr.reshape(NCH, len(FBLKS) * FBLK, CH)[:, :F] = xr_v
        x0bc = np.ascontiguousarray(np.broadcast_to(
            xr.reshape(NCH, len(FBLKS), 1, FBLK * CH),
            (NCH, len(FBLKS), 128, FBLK * CH)))
        # z0 for this core, per bd-chunk: [NCH, 128, K0C*CH]
        z0c = np.ascontiguousarray(
            z0p[sl].reshape(K0C, 128, NCH, CH).transpose(2, 1, 0, 3)
            .reshape(NCH, 128, K0C * CH))
        in_maps.append({
            "z0t": z0c, "x0b": x0bc, "w0": w0d, "w1": w1d, "w2": w2d,
            "bia": bia,
        })
    return in_maps


def _postprocess_core(res_c, fc_w):
    """res_c [128, NCH, NJ, NB_CH] fp32 -> per-core output [BC] (no fc_b)."""
    fcw4 = np.asarray(fc_w, dtype=np.float32).reshape(NJ, 128)   # [j, p]
    r = np.asarray(res_c, dtype=np.float32)
    return np.einsum('pcji,jp->ci', r, fcw4).reshape(BC)


def kernel(x, W0, b0, W1, b1, W2, b2, fc_w, fc_b):
    global LAST_RESULT
    if "nc" not in _CACHE:
        _CACHE["nc"] = _build_program()
    nc = _CACHE["nc"]

    in_maps = _prep_inputs(x, W0, b0, W1, b1, W2, b2, fc_w, fc_b)
    trace = bool(int(os.environ.get("NN_CIN_TRACE", "0")))
    res = run_bass_kernel_spmd(nc, in_maps, core_ids=list(range(NCORES)), trace=trace)
    LAST_RESULT = res

    fcb = float(np.asarray(fc_b, dtype=np.float32).reshape(-1)[0])
    out = np.empty((B, 1), dtype=np.float32)
    for c in range(NCORES):
        out[c * BC:(c + 1) * BC, 0] = _postprocess_core(
            res.results[c]["res"], fc_w) + fcb
    return out
